# revision 2
# baseline (speedup 1.0000x reference)
"""MinibatchDiscrimination kernel v5 for Trainium2 (8 NeuronCores, SPMD).

Problem:  x [256, 1024] f32, T [1024, 128, 32] f32
          M = einsum('ni,iok->nok', x, T)
          norm[a,b,o] = sum_k |M[a,o,k] - M[b,o,k]|
          o_b = exp(-norm).sum(axis=0) - 1            # [256, 128]
          out = concat([x, o_b], axis=1)              # [256, 1152]

Sharding: data-parallel over out_features; each core does 16 channels.

v5 design (vs the v4 baseline, |d| = 2*relu(d) - d as in v4):
  - 8 b's packed per PSUM norm tile (partitions = 16*b_idx + o, all 128
    used vs v4's 64), so one Exp covers 8 b's instead of 4.
  - block-triangular sweep: b-block j (32 b's) covers a in [32j, 256)
    only (FD = 256-32j); the (a < 32j, b) pairs are recovered from
    symmetry by a PE matmul that column-sums the exp tiles into
    obt[o, a-32], added on host.
  - per-FD DVE/ACT instruction split tuned to measured rates
    (DVE 2x_1P: (58+FD/2)/0.96 ns; ACT relu: (220+FD)/1.2 ns;
    ACT exp+accum: (300+2FD)/1.2 ns; PE MM: FD/2.4+23 ns).
"""

import os as _os_mod
_os_mod.environ["BASS_NEVER_TRACE"] = "1"

import numpy as np
import ml_dtypes

import concourse.bass as bass
import concourse.bacc as bacc
import concourse.mybir as mybir
import concourse.tile as tile
from concourse.bass_utils import run_bass_kernel_spmd

BF16 = ml_dtypes.bfloat16

N = 256          # batch
IN_F = 1024      # in features
OUT_F = 128      # out features (total)
K = 32           # kernel dim
NCORES = 8
O = OUT_F // NCORES   # out features per core (16)
NDG = N // 8          # dgrps of 8 b's (32)
BLK = 32              # triangle block (b's per FD step)
DG_PER_BLK = BLK // 8

# tiles-to-ACT per dgrp, keyed by FD; rest go to DVE
ACT_SPLIT = {256: 9, 224: 9, 192: 9, 160: 9, 128: 8, 96: 8, 64: 8, 32: 7}

# cw layout: [128, 8*128 + 16 + 16 + 128 + 64]
#   0:1024    bones2_8[b] (2.0 at [p, 16b + p//8])
#   1024:1040 sel16b8     (1.0 at [p, p%16])
#   1040:1056 bones1      (1.0 at [p, p//8])  -> ST row sums
#   1056:1184 negsel8     (-1.0 at [o, 16b+o], rows 0-15)
#   1184:1248 negselAB    (-1.0 at [o, o] cols 0-31; -1.0 at [o, 16+o] cols 32-64)
CW_COLS = 8 * 128 + 16 + 16 + 128 + 64

# DoubleRow stationary pair slots: (b0, b1) -> cwf8[:, slot]
PAIR_IDX = {(0, 1): 0, (2, 3): 1, (4, 5): 2, (6, 7): 3,
            (1, 2): 4, (3, 4): 5, (5, 6): 6}


def build_core_program(reps=1, act_split=None):
    import os as _os
    act_split = act_split or ACT_SPLIT

    nc = bacc.Bacc("TRN2", target_bir_lowering=False)

    xt_d = nc.dram_tensor("xt", [IN_F, N], mybir.dt.bfloat16, kind="ExternalInput")
    tsh_d = nc.dram_tensor("tsh", [IN_F, 512], mybir.dt.bfloat16, kind="ExternalInput")
    cw_d = nc.dram_tensor("cw", [128, CW_COLS], mybir.dt.bfloat16, kind="ExternalInput")
    use_dr = _os.environ.get("DR", "1") == "1"
    cwf8_d = None
    if use_dr:
        cwf8_d = nc.dram_tensor("cwf8", [128, 7, 2, 128], mybir.dt.float8e5,
                                kind="ExternalInput")
    ob_d = nc.dram_tensor("ob", [128, NDG], mybir.dt.float32, kind="ExternalOutput")
    obt_d = nc.dram_tensor("obt", [16, N - BLK], mybir.dt.float32, kind="ExternalOutput")

    with tile.TileContext(nc) as tc:
        with (
            tc.tile_pool(name="weights", bufs=1) as wpool,
            tc.tile_pool(name="mt", bufs=1) as mtpool,
            tc.tile_pool(name="absd", bufs=int(_os.environ.get("AD_BUFS", "10"))) as adpool,
            tc.tile_pool(name="escratch", bufs=int(_os.environ.get("E_BUFS", "3"))) as epool,
            tc.tile_pool(name="pdp", bufs=int(_os.environ.get("PD_BUFS", "5"))) as pdpool,
            tc.tile_pool(name="obp", bufs=1) as obpool,
        ):
            setup_psum_cm = tc.tile_pool(name="psum_mt", bufs=2, space=bass.MemorySpace.PSUM)
            pmt = setup_psum_cm.__enter__()
            psmall_cm = tc.tile_pool(name="psum_s", bufs=1, space=bass.MemorySpace.PSUM)
            psmall = psmall_cm.__enter__()

            # ---- load inputs ----
            cw = wpool.tile([128, CW_COLS], mybir.dt.bfloat16)
            nc.sync.dma_start(cw[:], cw_d[:])
            cwf8 = None
            if use_dr:
                cwf8 = wpool.tile([128, 7, 2, 128], mybir.dt.float8e5, tag="cwf8")
                nc.sync.dma_start(cwf8[:], cwf8_d[:])
            bones2_8 = [cw[:, 128 * b:128 * (b + 1)] for b in range(8)]
            sel16b8 = cw[:, 1024:1040]
            bones1 = cw[:, 1040:1056]
            negsel8 = cw[:16, 1056:1184]
            negselA = cw[:16, 1184:1216]
            negselB = cw[:16, 1216:1248]

            xtl = []
            tshl = []
            for it in range(8):
                xt_t = wpool.tile([128, N], mybir.dt.bfloat16, tag=f"xt{it}")
                nc.sync.dma_start(xt_t[:], xt_d[it * 128:(it + 1) * 128, :])
                xtl.append(xt_t)
                tsh_t = wpool.tile([128, 512], mybir.dt.bfloat16, tag=f"tsh{it}")
                nc.sync.dma_start(tsh_t[:], tsh_d[it * 128:(it + 1) * 128, :])
                tshl.append(tsh_t)

            # ---- MT = Tsh^T @ x^T : [(o,k), a] in 4 chunks of 128 partitions ----
            mt = []      # bf16 working copy
            mtf32 = []   # fp32 upcast of the bf16-rounded values (DVE scalar operand)
            nmt32 = []   # negated fp32 (ACT bias operand)
            for g in range(4):
                pm = pmt.tile([128, N], mybir.dt.float32)
                for it in range(8):
                    nc.tensor.matmul(
                        pm[:],
                        tshl[it][:, g * 128:(g + 1) * 128],
                        xtl[it][:],
                        start=(it == 0),
                        stop=(it == 7),
                    )
                mt_g = mtpool.tile([128, N], mybir.dt.bfloat16, tag=f"mt{g}")
                nc.vector.tensor_copy(mt_g[:], pm[:])
                # fp32 copies MUST come from the bf16 tile so values match exactly
                mt32_g = mtpool.tile([128, N], mybir.dt.float32, tag=f"mt32{g}")
                nc.vector.tensor_copy(mt32_g[:], mt_g[:])
                nm_g = mtpool.tile([128, N], mybir.dt.float32, tag=f"nmt32{g}")
                nc.scalar.activation(
                    nm_g[:], mt_g[:], mybir.ActivationFunctionType.Copy, scale=-1.0,
                )
                mt.append(mt_g)
                mtf32.append(mt32_g)
                nmt32.append(nm_g)

            # ---- ST[o, a] = sum_k MT ----
            st_ps = psmall.tile([16, N], mybir.dt.float32, tag="st_ps")
            for g in range(4):
                nc.tensor.matmul(
                    st_ps[:], bones1[:], mt[g][:], start=(g == 0), stop=(g == 3)
                )
            st_bf = mtpool.tile([16, N], mybir.dt.bfloat16, tag="st_bf")
            nc.vector.tensor_copy(st_bf[:], st_ps[:])

            # ---- bias tile: negsb[16*b + o, dg] = -ST[o, 8*dg + b] ----
            nsb_ps = psmall.tile([128, NDG], mybir.dt.float32, tag="nsb_ps")
            for j in range(4):
                nc.tensor.matmul(
                    nsb_ps[32 * j:32 * (j + 1), :],
                    negselA[:],
                    st_bf[:, 2 * j::8],
                    start=True,
                    stop=False,
                    tile_position=(0, 32 * j),
                )
                nc.tensor.matmul(
                    nsb_ps[32 * j:32 * (j + 1), :],
                    negselB[:],
                    st_bf[:, 2 * j + 1::8],
                    start=False,
                    stop=True,
                    tile_position=(0, 32 * j),
                )
            negsb = obpool.tile([128, NDG], mybir.dt.float32, tag="negsb")
            nc.vector.tensor_copy(negsb[:], nsb_ps[:])

            ob_acc = obpool.tile([128, NDG], mybir.dt.float32)

            psmall_cm.__exit__(None, None, None)
            setup_psum_cm.__exit__(None, None, None)
            pnorm_cm = tc.tile_pool(
                name="psum_norm",
                bufs=int(_os.environ.get("PNORM_BUFS", "6")),
                space=bass.MemorySpace.PSUM,
            )
            pnorm = pnorm_cm.__enter__()
            obt_cm = tc.tile_pool(name="psum_obt", bufs=1, space=bass.MemorySpace.PSUM)
            obt_pool = obt_cm.__enter__()
            obt_ps = obt_pool.tile([16, N - BLK], mybir.dt.float32, tag="obt")

            # ---- pairwise: dgrps of 8 b's, block-triangular ----
            import contextlib
            rep_ctx = tc.For_i(0, reps, 1) if reps > 1 else contextlib.nullcontext()

            # pending exp work, emitted one dgrp later so ACT's strict FIFO
            # is not blocked on the PE accumulation chain
            pending = []

            def flush_pending():
                for nt_, FD_, dg_, blk_ in pending:
                    e = epool.tile([128, N], mybir.dt.bfloat16, tag="e")
                    nc.scalar.activation(
                        e[:, :FD_], nt_[:, :FD_], mybir.ActivationFunctionType.Exp,
                        scale=-1.0,
                        bias=negsb[:, dg_:dg_ + 1],
                        accum_out=ob_acc[:, dg_:dg_ + 1],
                    )
                    if FD_ > BLK:
                        # columns a >= BLK*(blk+1): transpose-accumulate into
                        # obt[o, a-BLK]; the (a < 32j, b) pairs of later
                        # blocks come from symmetry
                        nc.tensor.matmul(
                            obt_ps[:, BLK * blk_:],
                            sel16b8,
                            e[:, BLK:FD_],
                            start=(dg_ == first_obt_dg),
                            stop=(dg_ == last_obt_dg),
                            skip_group_check=True,
                        )
                pending.clear()

            flush_at = int(_os.environ.get("FLUSH_AT", "16"))
            dg_order = list(range(NDG))
            if _os.environ.get("DG_ORDER", "inter") == "inter":
                half = NDG // 2
                dg_order = [x for p in zip(range(half), range(half, NDG)) for x in p]
            obt_dgs = [d for d in dg_order if (N - BLK * (d // DG_PER_BLK)) > BLK]
            first_obt_dg = obt_dgs[0]
            last_obt_dg = obt_dgs[-1]
            with rep_ctx:
                for dg in dg_order:
                    blk = dg // DG_PER_BLK
                    a0 = BLK * blk
                    FD = N - a0
                    n_act = act_split[FD]

                    # norm tiles are allocated at full width so the PSUM pool
                    # keeps a single tile geometry; only [:, :FD] is used
                    nt = pnorm.tile([128, N], mybir.dt.float32, tag="nt")
                    # -ST[o, a] into all 8 bands (rank-1 part of |d|)
                    nc.tensor.matmul(
                        nt[:, :FD], negsel8, st_bf[:, a0:], start=True, stop=False,
                    )

                    # ACT tile placement pattern
                    kinds = ["dve"] * 32
                    act_pos = _os.environ.get("ACT_POS", "spread")
                    if act_pos == "tail":
                        for i in range(32 - n_act, 32):
                            kinds[i] = "act"
                    elif act_pos == "head":
                        for i in range(n_act):
                            kinds[i] = "act"
                    else:
                        acc = 0
                        for i in range(32):
                            acc += n_act
                            if acc >= 32:
                                acc -= 32
                                kinds[i] = "act"

                    # fp8 DoubleRow pairing of ACT tiles (consecutive in
                    # emission order); PAIR_IDX maps (b0, b1) -> cwf8 slot
                    act_is = [i for i in range(32) if kinds[i] == "act"]
                    pair_of = {}
                    pair_slot = {}
                    if use_dr:
                        for pj in range(len(act_is) // 2):
                            i1, i2 = act_is[2 * pj], act_is[2 * pj + 1]
                            key = (i1 // 4, i2 // 4)
                            if key in PAIR_IDX:
                                pair_of[i1] = ("first", i2)
                                pair_of[i2] = ("second", i1)
                                pair_slot[i2] = PAIR_IDX[key]

                    half_flushed = False
                    pd = None
                    for b_idx in range(8):
                        b = 8 * dg + b_idx
                        for g in range(4):
                            i = 4 * b_idx + g
                            if i == flush_at and not half_flushed:
                                flush_pending()
                                half_flushed = True
                            role = pair_of.get(i, (None, None))[0]
                            if role is not None:
                                if role == "first":
                                    pd = pdpool.tile([128, 2, N], mybir.dt.float8e5, tag="pd")
                                    nc.scalar.activation(
                                        pd[:, 0, :FD], mt[g][:, a0:],
                                        mybir.ActivationFunctionType.Relu,
                                        bias=nmt32[g][:, b:b + 1],
                                    )
                                else:
                                    nc.scalar.activation(
                                        pd[:, 1, :FD], mt[g][:, a0:],
                                        mybir.ActivationFunctionType.Relu,
                                        bias=nmt32[g][:, b:b + 1],
                                    )
                                    nc.tensor.matmul(
                                        nt[:, :FD],
                                        cwf8[:, pair_slot[i]],
                                        pd[:, :, :FD],
                                        start=False,
                                        stop=(i == 31),
                                        perf_mode=mybir.MatmulPerfMode.DoubleRow,
                                    )
                                continue
                            ad = adpool.tile([128, N], mybir.dt.bfloat16, tag="ad")
                            if kinds[i] == "act":
                                nc.scalar.activation(
                                    ad[:, :FD], mt[g][:, a0:],
                                    mybir.ActivationFunctionType.Relu,
                                    bias=nmt32[g][:, b:b + 1],
                                )
                            else:
                                nc.vector.tensor_scalar(
                                    ad[:, :FD], mt[g][:, a0:], mtf32[g][:, b:b + 1], 0.0,
                                    mybir.AluOpType.subtract, mybir.AluOpType.max,
                                )
                            nc.tensor.matmul(
                                nt[:, :FD],
                                bones2_8[b_idx],
                                ad[:, :FD],
                                start=False,
                                stop=(i == 31),
                            )
                    pending.append((nt, FD, dg, blk))
                flush_pending()

            obt_cm.__exit__(None, None, None)
            pnorm_cm.__exit__(None, None, None)
            obt_sb = obpool.tile([16, N - BLK], mybir.dt.float32, tag="obt_sb")
            nc.vector.tensor_copy(obt_sb[:], obt_ps[:])
            nc.sync.dma_start(obt_d[:], obt_sb[:])
            ob_final = obpool.tile([128, NDG], mybir.dt.float32)
            nc.vector.tensor_scalar_add(ob_final[:], ob_acc[:], -1.0)
            nc.sync.dma_start(ob_d[:], ob_final[:])

    nc.compile()
    return nc


F8E5 = ml_dtypes.float8_e5m2


def host_prep_shared(x):
    xt = np.ascontiguousarray(x.T).astype(BF16)
    cwf8 = np.zeros((128, 7, 2, 128), dtype=F8E5)
    for (b0, b1), slot in PAIR_IDX.items():
        for p in range(128):
            cwf8[p, slot, 0, 16 * b0 + p // 8] = 2.0
            cwf8[p, slot, 1, 16 * b1 + p // 8] = 2.0
    cw = np.zeros((128, CW_COLS), dtype=BF16)
    for b in range(8):
        for p in range(128):
            cw[p, 128 * b + 16 * b + p // 8] = 2.0       # bones2_8[b]
    for p in range(128):
        cw[p, 1024 + (p % 16)] = 1.0                     # sel16b8
        cw[p, 1040 + p // 8] = 1.0                       # bones1
    for o in range(16):
        for b in range(8):
            cw[o, 1056 + 16 * b + o] = -1.0              # negsel8
        cw[o, 1184 + o] = -1.0                           # negselA
        cw[o, 1216 + 16 + o] = -1.0                      # negselB
    return xt, cw, cwf8


def pack_tsh(T_core):
    """T_core [IN_F, O, K] -> [IN_F, 512] with col = g*128 + o*8 + k_l, k = 8g + k_l."""
    return np.ascontiguousarray(
        T_core.reshape(IN_F, O, 4, 8).transpose(0, 2, 1, 3).reshape(IN_F, 512)
    ).astype(BF16)


def unscramble(ob_raw, obt_raw):
    """ob_raw [128, NDG] f32, obt [16, N-BLK] -> [N, O].

    ob_raw[16*b_idx + o, dg] = direct sum for n = 8*dg + b_idx (a >= 32*blk).
    obt[o, a-BLK] = symmetric sum over earlier blocks' b's for sample a.
    """
    a = np.asarray(ob_raw).reshape(8, 16, NDG)  # [b_idx, o, dg]
    ob = a.transpose(2, 0, 1).reshape(N, 16).copy()  # [n, o]
    ob[BLK:, :] += np.asarray(obt_raw).T        # [a-BLK, o]
    return ob


_NC_CACHE = None


def kernel(x, T):
    global _NC_CACHE
    x = np.asarray(x, dtype=np.float32)
    T = np.asarray(T, dtype=np.float32)
    assert x.shape == (N, IN_F) and T.shape == (IN_F, OUT_F, K)

    if _NC_CACHE is None:
        _NC_CACHE = build_core_program()
    nc = _NC_CACHE

    xt, cw, cwf8 = host_prep_shared(x)
    in_maps = []
    for c in range(NCORES):
        tsh = pack_tsh(T[:, c * O:(c + 1) * O, :])
        in_maps.append({"xt": xt, "tsh": tsh, "cw": cw, "cwf8": cwf8})

    res = run_bass_kernel_spmd(nc, in_maps, core_ids=list(range(NCORES)))

    cores = []
    for r in res.results:
        cores.append(unscramble(r["ob"], r["obt"]))
    ob = np.concatenate(cores, axis=1).astype(np.float32)

    out = np.empty((N, IN_F + OUT_F), dtype=np.float32)
    out[:, :IN_F] = x
    out[:, IN_F:] = ob
    return out
